# revision 16
# baseline (speedup 1.0000x reference)
"""Trainium2 Bass kernel for nn_Attention2d (N=32, C=128, S=32*36=1152, OUT=5000).

Math (per image i):
    xe = x.reshape(C,S) + pos                      # (C,S)
    scores[s,n] = sum_c xe[c,s] * nq[c,n]          # QK, contraction over C
    attn = softmax_s(scores)
    y[n] = sum_c f[c,n] * sum_s x[c,s]*attn[s,n] + bias[n]

Reformulation used here (all big matmuls contract over C=128 = partition dim):
    Z[s,n]   = sum_c x[c,s] * f[c,n]               # same shape/layout as scores
    E        = exp(scores)                          # no max-subtraction needed
    num[n]   = sum_s E[s,n] * Z[s,n]
    den[n]   = sum_s E[s,n]
    y[n]     = num[n]/den[n] + bias[n]

Layout: transposed [n_partition, s_free] tiles so that:
  - scoresT/ZT chunks come from matmuls lhsT=nq/f[:, nchunk(128)], rhs=xe/x[:, s]
  - exp runs on ACT with accum_out -> den  (free-dim = s reduction)
  - num comes from fused DVE passes: scalar_tensor_tensor(E * ZT, accum=sum_s)
    (TensorTensorReduce crashes the DVE on this silicon; STT's accum works.)

Pipeline: the 8 PSUM banks are the scarce resource. scoresT tiles [128,1152]
(3 banks) are double-buffered (6 banks) so the next image's QK matmuls overlap
the current exp; ZT streams through a 2-slot ring of 1-bank [128,512] tiles,
with the fused multiply split into 3 sub-passes whose partial sums land in
separate accumulator columns merged at image end. The loop is chunk-outer/
image-inner (weight loads amortize over 4 images) and Z/STT work is emitted
with a 1-image lag so the in-order PE alternates S-bursts with Z pieces.
Inputs are cast to bf16 on the host (halves DMA, enables 1-cyc/row matmuls).
xe = x + pos is precomputed on the HOST (removes 4 DVE adds + the pos DMA
from the startup critical path; DVE runs only the STT stream + tiny epilogue
pieces). Startup DMAs are split and spread across engine queues with xe0
first so chunk 0's QK can start ASAP. Per-image epilogues run pipelined with
the last chunk's compute; the y1=num*rcp and y=y1+bias pieces run on the
otherwise-idle GPSIMD so they don't interrupt the DVE STT stream; the output
is padded to 5120 so each image writes with ONE DMA. Z/STT pieces are a
uniform 384 wide: uniform pieces equalize the DVE inter-piece window with
the just-in-time Z-matmul chain (ring depth 2 is bank-limited). Engine busy
at baseline (287.7us): DVE 255 / ACT 225 / PE 216+90ldw; DVE's STT stream
(244.8us) is the PSUM-geometry-locked floor, so the wins here are
head/tail/stream-interruption trims. fp8 DoubleRow matmuls were evaluated
and are numerically DEAD for this problem (rel err 3-4e-2 > 2e-2 gate, both
for QK-only and Z-only fp8; bf16 gives 2.9e-3).

Sharding: batch N=32 across 8 cores (4 images/core), no collectives.
"""

import os
import sys

for _p in ("/opt/trn_rl_repo", "/root/.axon_site/_ro/trn_rl_repo"):
    if os.path.isdir(_p) and _p not in sys.path:
        sys.path.append(_p)

import ml_dtypes
import numpy as np

BF16 = ml_dtypes.bfloat16

N, C, W, H = 32, 128, 32, 36
S = W * H          # 1152
OUT = 5000
CORES = 8
IPC = N // CORES   # images per core = 4
NCH = 40           # n-chunks of 128 partitions (OUT padded to 5120)
OUTP = NCH * 128   # 5120

S_SLICES = [(0, 512), (512, 1024), (1024, 1152)]
# uniform Z/STT pieces: equalizes the DVE inter-piece window with the
# just-in-time Z-matmul chain (512/512/128 made every 512-piece wait ~240ns)
Z_SLICES = [(0, 384), (384, 768), (768, 1152)]

_CACHE = {}


def _build_nc():
    import concourse.tile as tile
    from concourse import bacc, mybir
    from concourse.masks import make_identity

    f32 = mybir.dt.float32
    bf16 = mybir.dt.bfloat16
    EXP = mybir.ActivationFunctionType.Exp
    MULT = mybir.AluOpType.mult
    ADD = mybir.AluOpType.add
    X = mybir.AxisListType.X

    nc = bacc.Bacc()

    SUB = mybir.AluOpType.subtract

    xe_d = nc.dram_tensor("xe", [IPC, C, S], bf16, kind="ExternalInput")
    pos_d = nc.dram_tensor("pos", [C, S], bf16, kind="ExternalInput")
    nq_d = nc.dram_tensor("neuron_query", [C, OUTP], bf16, kind="ExternalInput")
    f_d = nc.dram_tensor("features", [C, OUTP], bf16, kind="ExternalInput")
    b_d = nc.dram_tensor("bias", [128, NCH], f32, kind="ExternalInput")
    o_d = nc.dram_tensor("out", [IPC, OUTP], f32, kind="ExternalOutput")

    with tile.TileContext(nc) as tc:
        with (
            tc.tile_pool(name="singles", bufs=1) as singles,
            tc.tile_pool(name="imgs", bufs=IPC) as imgs_pool,
            tc.tile_pool(name="accs", bufs=2 * IPC) as acc_pool,
            tc.tile_pool(name="epool", bufs=6) as e_pool,
            tc.tile_pool(name="scpool", bufs=2) as sc_pool,
            tc.tile_pool(name="epi", bufs=2) as epi_pool,
            tc.tile_pool(name="psS", bufs=2, space="PSUM") as psS,
            tc.tile_pool(name="psZ", bufs=2, space="PSUM") as psZ,
        ):
            # ---- one-time loads ----
            # Queue plan: only xe is shipped per image (xf = xe - pos is
            # recomputed on the otherwise-idle GPSIMD, keeping the image DMA
            # traffic at 1 tensor/image). sync: xe0 first (gates chunk 0),
            # then nq pieces + pos + bias; scalar: xe1-3; gpsimd queue: f
            # pieces (issued before the Pool TTs so transfers overlap them).
            xe_l, xf_l, num3_l, den_l = [], [], [], []
            for i in range(IPC):
                xe_t = imgs_pool.tile([C, S], bf16, tag="xe")
                xf_t = imgs_pool.tile([C, S], bf16, tag="xf")
                xe_l.append(xe_t)
                xf_l.append(xf_t)
                num3_t = acc_pool.tile([128, NCH * 3], f32, tag="num3")
                den_t = acc_pool.tile([128, NCH], f32, tag="den")
                num3_l.append(num3_t)
                den_l.append(den_t)

            # Only sync(SP)/scalar(ACT)/gpsimd queues can issue DMAs. xe0
            # gates the stream start: split it by partitions across sync and
            # scalar so both halves transfer in parallel; nq0 rides first on
            # sync (QK chunk 0 needs it), f pieces on gpsimd. With the
            # image-major main loop below, xe1-3 have ~60us of slack each.
            pos_t = singles.tile([C, S], bf16, tag="pos")
            nc.scalar.dma_start(out=pos_t, in_=pos_d[:, :])

            PIECES = [256, 768, 1536, 2560]  # cols per piece, sum=OUTP
            nq_tiles, f_tiles = [], []
            piece_of = []  # chunk -> (piece idx, col offset)
            lo = 0
            for pi, w in enumerate(PIECES):
                nq_p = singles.tile([C, w], bf16, tag=f"nq{pi}")
                f_p = singles.tile([C, w], bf16, tag=f"f{pi}")
                nc.gpsimd.dma_start(out=f_p, in_=f_d[:, lo : lo + w])
                nq_tiles.append(nq_p)
                f_tiles.append(f_p)
                for c in range(lo // 128, (lo + w) // 128):
                    piece_of.append((pi, c * 128 - lo))
                lo += w

            nc.sync.dma_start(out=nq_tiles[0], in_=nq_d[:, 0:PIECES[0]])
            nc.sync.dma_start(out=xe_l[0][0:64, :], in_=xe_d[0, 0:64, :])
            nc.scalar.dma_start(out=xe_l[0][64:128, :], in_=xe_d[0, 64:128, :])
            lo = PIECES[0]
            for pi in range(1, len(PIECES)):
                nc.sync.dma_start(
                    out=nq_tiles[pi], in_=nq_d[:, lo : lo + PIECES[pi]]
                )
                lo += PIECES[pi]
            for i in range(1, IPC):
                nc.scalar.dma_start(out=xe_l[i], in_=xe_d[i])

            bias_t = singles.tile([128, NCH], f32)
            nc.sync.dma_start(out=bias_t, in_=b_d[:, :])

            # xf_i = xe_i - pos: xf0 on the DVE (which idles until the first
            # STT anyway, and Pool's DMA issues would delay it past Z(0)'s
            # deadline); xf1-3 on the Pool engine with ~60us of slack each.
            nc.vector.tensor_tensor(xf_l[0], xe_l[0], pos_t, op=SUB)
            for i in range(1, IPC):
                nc.gpsimd.tensor_tensor(xf_l[i], xe_l[i], pos_t, op=SUB)

            ident = singles.tile([128, 128], f32)
            make_identity(nc, ident)

            # ---- main loop: chunk-outer, image-inner ----
            # Z/STT pieces for each image are emitted with a 1-image lag so
            # the in-order PE interleaves S-bursts with Z pieces and each
            # STT's inputs (exp output + Z matmul) are ready when it issues.
            def emit_epilogue(i, on_dve=False):
                # on_dve: the final image's epilogue is latency-critical
                # (nothing left to overlap) — keep it on the DVE to skip the
                # DVE->Pool->PE sem hops; mid-stream epilogues use the Pool
                # so they don't interrupt the STT stream.
                rcp_t = epi_pool.tile([128, NCH], f32, tag="rcp")
                nc.vector.reciprocal(out=rcp_t, in_=den_l[i])
                num_t = epi_pool.tile([128, NCH], f32, tag="num")
                nc.vector.tensor_reduce(
                    out=num_t,
                    in_=num3_l[i].rearrange("p (c j) -> p c j", j=3),
                    op=ADD,
                    axis=X,
                )
                mul_eng = nc.vector if on_dve else nc.gpsimd
                y1_t = epi_pool.tile([128, NCH], f32, tag="y1")
                mul_eng.tensor_mul(y1_t, num_t, rcp_t)
                y_t = epi_pool.tile([128, NCH], f32, tag="y")
                mul_eng.tensor_add(y_t, y1_t, bias_t)

                pt = psZ.tile([NCH, 128], f32, tag="z")
                nc.tensor.transpose(out=pt, in_=y_t, identity=ident)
                yT_t = epi_pool.tile([NCH, 128], f32, tag="yT")
                nc.scalar.copy(out=yT_t, in_=pt)

                nc.sync.dma_start(
                    out=o_d[i].rearrange("(c p) -> c p", p=128),
                    in_=yT_t,
                )

            pending = None  # (e_t, img, chunk, f_c)

            def emit_z(e_t, i, c, f_c):
                for j, (lo, hi) in enumerate(Z_SLICES):
                    w = hi - lo
                    z_t = psZ.tile([128, 512], f32, tag="z")
                    nc.tensor.matmul(
                        z_t[:, 0:w], f_c, xf_l[i][:, lo:hi],
                        start=True, stop=True,
                    )
                    sc_t = sc_pool.tile([128, 512], f32, tag="sc")
                    col = c * 3 + j
                    nc.vector.scalar_tensor_tensor(
                        out=sc_t[:, 0:w],
                        in0=e_t[:, lo:hi],
                        scalar=1.0,
                        in1=z_t[:, 0:w],
                        op0=MULT,
                        op1=MULT,
                        accum_out=num3_l[i][:, col : col + 1],
                    )

            # image-major: image i's chunks run back-to-back, so only xe0
            # gates the stream start and epilogues/output DMAs for images
            # 0-2 overlap the stream instead of piling up at the end.
            # Each epilogue is DELAYED a few units past its image boundary:
            # emitted at the boundary, its PE-transpose gets scheduled ahead
            # of Z(39,i) on the in-order PE while depending on STT(39,i)
            # through the Pool chain — the near-circular wait stretches the
            # boundary ~2-3us. A few units later everything it needs is done
            # and its pieces slot into per-unit engine slack.
            EPI_DELAY = 8
            epi_queue = []  # (due_unit_idx, image)
            unit = 0
            for i in range(IPC):
                for c in range(NCH):
                    pi, po = piece_of[c]
                    nq_c = nq_tiles[pi][:, po : po + 128]
                    f_c = f_tiles[pi][:, po : po + 128]
                    s_t = psS.tile([128, S], f32, tag="s")
                    for lo, hi in S_SLICES:
                        nc.tensor.matmul(
                            s_t[:, lo:hi], nq_c, xe_l[i][:, lo:hi],
                            start=True, stop=True,
                        )
                    e_t = e_pool.tile([128, S], bf16, tag="e")
                    nc.scalar.activation(
                        out=e_t, in_=s_t, func=EXP,
                        accum_out=den_l[i][:, c : c + 1],
                    )
                    if pending is not None:
                        emit_z(*pending)
                    if c == NCH - 1:
                        # last chunk of image i: emit its Z/STT immediately
                        # (no 1-unit lag) so no cross-image PE dependency
                        # chain spans the boundary
                        emit_z(e_t, i, c, f_c)
                        epi_queue.append((unit + EPI_DELAY, i))
                        pending = None
                    else:
                        pending = (e_t, i, c, f_c)
                    if epi_queue and unit >= epi_queue[0][0]:
                        emit_epilogue(epi_queue.pop(0)[1])
                    unit += 1
            for _, img in epi_queue:
                emit_epilogue(img, on_dve=True)

    nc.compile()
    return nc


def _get_nc():
    if "nc" not in _CACHE:
        _CACHE["nc"] = _build_nc()
    return _CACHE["nc"]


def _prep_in_maps(inputs):
    xf32 = np.asarray(inputs["x"], dtype=np.float32).reshape(N, C, S)
    pos32 = np.asarray(inputs["pos_emb"], dtype=np.float32).reshape(C, S)
    pos = pos32.astype(BF16)
    # On-device xf = xe - pos (bf16 subtract on GPSIMD) recovers bf16(x)
    # up to one extra bf16 rounding — well inside the error budget.
    xe = np.ascontiguousarray(
        (xf32.astype(BF16).astype(np.float32) + pos.astype(np.float32)).astype(
            BF16
        )
    )
    nqp = np.zeros((C, OUTP), BF16)
    nqp[:, :OUT] = (
        np.asarray(inputs["neuron_query"], dtype=np.float32)
        .reshape(C, OUT)
        .astype(BF16)
    )
    ftp = np.zeros((C, OUTP), BF16)
    ftp[:, :OUT] = (
        np.asarray(inputs["features"], dtype=np.float32)
        .reshape(C, OUT)
        .astype(BF16)
    )
    bias_pad = np.zeros(OUTP, np.float32)
    bias_pad[:OUT] = np.asarray(inputs["bias"], dtype=np.float32)
    # biasp[p, c] = bias_pad[c*128 + p]
    biasp = np.ascontiguousarray(bias_pad.reshape(NCH, 128).T)
    return [
        {
            "xe": np.ascontiguousarray(xe[i * IPC : (i + 1) * IPC]),
            "pos": pos,
            "neuron_query": nqp,
            "features": ftp,
            "bias": biasp,
        }
        for i in range(CORES)
    ]


def run_kernel(inputs, trace=False):
    """Returns (out [N, OUT] float32, BassKernelResults)."""
    from concourse.bass_utils import run_bass_kernel_spmd

    nc = _get_nc()
    in_maps = _prep_in_maps(inputs)
    res = run_bass_kernel_spmd(nc, in_maps, list(range(CORES)), trace=trace)
    out = np.concatenate([r["out"][:, :OUT] for r in res.results], axis=0)
    return np.asarray(out, dtype=np.float32), res


def kernel(**inputs):
    out, _ = run_kernel(inputs, trace=False)
    return out


# revision 18
# speedup vs baseline: 1.0242x; 1.0242x over previous
"""Trainium2 Bass kernel for nn_Attention2d (N=32, C=128, S=32*36=1152, OUT=5000).

Math (per image i):
    xe = x.reshape(C,S) + pos                      # (C,S)
    scores[s,n] = sum_c xe[c,s] * nq[c,n]          # QK, contraction over C
    attn = softmax_s(scores)
    y[n] = sum_c f[c,n] * sum_s x[c,s]*attn[s,n] + bias[n]

Reformulation used here (all big matmuls contract over C=128 = partition dim):
    Z[s,n]   = sum_c x[c,s] * f[c,n]               # same shape/layout as scores
    E        = exp(scores)                          # no max-subtraction needed
    num[n]   = sum_s E[s,n] * Z[s,n]
    den[n]   = sum_s E[s,n]
    y[n]     = num[n]/den[n] + bias[n]

Layout: transposed [n_partition, s_free] tiles so that:
  - scoresT/ZT chunks come from matmuls lhsT=nq/f[:, nchunk(128)], rhs=xe/x[:, s]
  - exp runs on ACT with accum_out -> den  (free-dim = s reduction)
  - num comes from fused DVE passes: scalar_tensor_tensor(E * ZT, accum=sum_s)
    (TensorTensorReduce crashes the DVE on this silicon; STT's accum works.)

Pipeline: the 8 PSUM banks are the scarce resource. scoresT tiles [128,1152]
(3 banks) are double-buffered (6 banks) so the next image's QK matmuls overlap
the current exp; ZT streams through a 2-slot ring of 1-bank [128,512] tiles,
with the fused multiply split into 3 sub-passes whose partial sums land in
separate accumulator columns merged at image end. The loop is chunk-outer/
image-inner (weight loads amortize over 4 images) and Z/STT work is emitted
with a 1-image lag so the in-order PE alternates S-bursts with Z pieces.
Inputs are cast to bf16 on the host (halves DMA, enables 1-cyc/row matmuls).
xe = x + pos is precomputed on the HOST (removes 4 DVE adds + the pos DMA
from the startup critical path; DVE runs only the STT stream + tiny epilogue
pieces). Startup DMAs are split and spread across engine queues with xe0
first so chunk 0's QK can start ASAP. Per-image epilogues run pipelined with
the last chunk's compute; the y1=num*rcp and y=y1+bias pieces run on the
otherwise-idle GPSIMD so they don't interrupt the DVE STT stream; the output
is padded to 5120 so each image writes with ONE DMA. Z/STT pieces are a
uniform 384 wide: uniform pieces equalize the DVE inter-piece window with
the just-in-time Z-matmul chain (ring depth 2 is bank-limited). Engine busy
at baseline (287.7us): DVE 255 / ACT 225 / PE 216+90ldw; DVE's STT stream
(244.8us) is the PSUM-geometry-locked floor, so the wins here are
head/tail/stream-interruption trims. fp8 DoubleRow matmuls were evaluated
and are numerically DEAD for this problem (rel err 3-4e-2 > 2e-2 gate, both
for QK-only and Z-only fp8; bf16 gives 2.9e-3).

Sharding: batch N=32 across 8 cores (4 images/core), no collectives.
"""

import os
import sys

for _p in ("/opt/trn_rl_repo", "/root/.axon_site/_ro/trn_rl_repo"):
    if os.path.isdir(_p) and _p not in sys.path:
        sys.path.append(_p)

import ml_dtypes
import numpy as np

BF16 = ml_dtypes.bfloat16

N, C, W, H = 32, 128, 32, 36
S = W * H          # 1152
OUT = 5000
CORES = 8
IPC = N // CORES   # images per core = 4
NCH = 40           # n-chunks of 128 partitions (OUT padded to 5120)
OUTP = NCH * 128   # 5120

S_SLICES = [(0, 512), (512, 1024), (1024, 1152)]
# uniform Z/STT pieces: equalizes the DVE inter-piece window with the
# just-in-time Z-matmul chain (512/512/128 made every 512-piece wait ~240ns)
Z_SLICES = [(0, 384), (384, 768), (768, 1152)]

_CACHE = {}


def _build_nc():
    import concourse.tile as tile
    from concourse import bacc, mybir
    from concourse.masks import make_identity

    f32 = mybir.dt.float32
    bf16 = mybir.dt.bfloat16
    EXP = mybir.ActivationFunctionType.Exp
    MULT = mybir.AluOpType.mult
    ADD = mybir.AluOpType.add
    X = mybir.AxisListType.X

    nc = bacc.Bacc()

    SUB = mybir.AluOpType.subtract

    xe_d = nc.dram_tensor("xe", [IPC, C, S], bf16, kind="ExternalInput")
    pos_d = nc.dram_tensor("pos", [C, S], bf16, kind="ExternalInput")
    nq_d = nc.dram_tensor("neuron_query", [C, OUTP], bf16, kind="ExternalInput")
    f_d = nc.dram_tensor("features", [C, OUTP], bf16, kind="ExternalInput")
    b_d = nc.dram_tensor("bias", [128, NCH], f32, kind="ExternalInput")
    o_d = nc.dram_tensor("out", [IPC, OUTP], f32, kind="ExternalOutput")

    with tile.TileContext(nc) as tc:
        with (
            tc.tile_pool(name="singles", bufs=1) as singles,
            tc.tile_pool(name="imgs", bufs=IPC) as imgs_pool,
            tc.tile_pool(name="accs", bufs=2 * IPC) as acc_pool,
            tc.tile_pool(name="epool", bufs=6) as e_pool,
            tc.tile_pool(name="scpool", bufs=2) as sc_pool,
            tc.tile_pool(name="epi", bufs=2) as epi_pool,
            tc.tile_pool(name="psS", bufs=2, space="PSUM") as psS,
            tc.tile_pool(name="psZ", bufs=2, space="PSUM") as psZ,
        ):
            # ---- one-time loads ----
            # Queue plan: only xe is shipped per image (xf = xe - pos is
            # recomputed on the otherwise-idle GPSIMD, keeping the image DMA
            # traffic at 1 tensor/image). sync: xe0 first (gates chunk 0),
            # then nq pieces + pos + bias; scalar: xe1-3; gpsimd queue: f
            # pieces (issued before the Pool TTs so transfers overlap them).
            xe_l, xf_l, num3_l, den_l = [], [], [], []
            for i in range(IPC):
                xe_t = imgs_pool.tile([C, S], bf16, tag="xe")
                xf_t = imgs_pool.tile([C, S], bf16, tag="xf")
                xe_l.append(xe_t)
                xf_l.append(xf_t)
                num3_t = acc_pool.tile([128, NCH * 3], f32, tag="num3")
                den_t = acc_pool.tile([128, NCH], f32, tag="den")
                num3_l.append(num3_t)
                den_l.append(den_t)

            # Only sync(SP)/scalar(ACT)/gpsimd queues can issue DMAs. xe0
            # gates the stream start: split it by partitions across sync and
            # scalar so both halves transfer in parallel; nq0 rides first on
            # sync (QK chunk 0 needs it), f pieces on gpsimd. With the
            # image-major main loop below, xe1-3 have ~60us of slack each.
            pos_t = singles.tile([C, S], bf16, tag="pos")
            nc.scalar.dma_start(out=pos_t, in_=pos_d[:, :])

            PIECES = [256, 768, 1536, 2560]  # cols per piece, sum=OUTP
            nq_tiles, f_tiles = [], []
            piece_of = []  # chunk -> (piece idx, col offset)
            lo = 0
            for pi, w in enumerate(PIECES):
                nq_p = singles.tile([C, w], bf16, tag=f"nq{pi}")
                f_p = singles.tile([C, w], bf16, tag=f"f{pi}")
                nc.gpsimd.dma_start(out=f_p, in_=f_d[:, lo : lo + w])
                nq_tiles.append(nq_p)
                f_tiles.append(f_p)
                for c in range(lo // 128, (lo + w) // 128):
                    piece_of.append((pi, c * 128 - lo))
                lo += w

            nc.sync.dma_start(out=nq_tiles[0], in_=nq_d[:, 0:PIECES[0]])
            nc.sync.dma_start(out=xe_l[0][0:64, :], in_=xe_d[0, 0:64, :])
            nc.scalar.dma_start(out=xe_l[0][64:128, :], in_=xe_d[0, 64:128, :])
            lo = PIECES[0]
            for pi in range(1, len(PIECES)):
                nc.sync.dma_start(
                    out=nq_tiles[pi], in_=nq_d[:, lo : lo + PIECES[pi]]
                )
                lo += PIECES[pi]
            for i in range(1, IPC):
                nc.scalar.dma_start(out=xe_l[i], in_=xe_d[i])

            bias_t = singles.tile([128, NCH], f32)
            nc.sync.dma_start(out=bias_t, in_=b_d[:, :])

            # xf_i = xe_i - pos: xf0 on the DVE (which idles until the first
            # STT anyway, and Pool's DMA issues would delay it past Z(0)'s
            # deadline); xf1-3 on the Pool engine with ~60us of slack each.
            nc.vector.tensor_tensor(xf_l[0], xe_l[0], pos_t, op=SUB)
            for i in range(1, IPC):
                nc.gpsimd.tensor_tensor(xf_l[i], xe_l[i], pos_t, op=SUB)

            ident = singles.tile([128, 128], f32)
            make_identity(nc, ident)

            # ---- main loop: chunk-outer, image-inner ----
            # Z/STT pieces for each image are emitted with a 1-image lag so
            # the in-order PE interleaves S-bursts with Z pieces and each
            # STT's inputs (exp output + Z matmul) are ready when it issues.
            y_tiles = {}

            def emit_epilogue_a(i, on_dve=False):
                # on_dve: the final image's epilogue is latency-critical
                # (nothing left to overlap) — keep it on the DVE to skip the
                # DVE->Pool sem hops; mid-stream epilogues use the Pool so
                # they don't interrupt the STT stream.
                rcp_t = epi_pool.tile([128, NCH], f32, tag="rcp")
                nc.vector.reciprocal(out=rcp_t, in_=den_l[i])
                num_t = epi_pool.tile([128, NCH], f32, tag="num")
                nc.vector.tensor_reduce(
                    out=num_t,
                    in_=num3_l[i].rearrange("p (c j) -> p c j", j=3),
                    op=ADD,
                    axis=X,
                )
                mul_eng = nc.vector if on_dve else nc.gpsimd
                y1_t = epi_pool.tile([128, NCH], f32, tag="y1")
                mul_eng.tensor_mul(y1_t, num_t, rcp_t)
                y_t = epi_pool.tile([128, NCH], f32, tag="y")
                mul_eng.tensor_add(y_t, y1_t, bias_t)
                y_tiles[i] = y_t

            def emit_epilogue_b(i):
                # stage b runs a couple of units after stage a, so y is ready
                # by the time the transpose's psZ ring turn comes — a
                # transpose waiting on y parks in a ring slot and starves
                # the Z matmuls (1.3us DVE gap per epilogue otherwise)
                pt = psZ.tile([NCH, 128], f32, tag="z")
                nc.tensor.transpose(out=pt, in_=y_tiles[i], identity=ident)
                yT_t = epi_pool.tile([NCH, 128], f32, tag="yT")
                nc.scalar.copy(out=yT_t, in_=pt)

                nc.sync.dma_start(
                    out=o_d[i].rearrange("(c p) -> c p", p=128),
                    in_=yT_t,
                )

            pending = None  # (e_t, img, chunk, f_c)

            def emit_z(e_t, i, c, f_c):
                for j, (lo, hi) in enumerate(Z_SLICES):
                    w = hi - lo
                    z_t = psZ.tile([128, 512], f32, tag="z")
                    nc.tensor.matmul(
                        z_t[:, 0:w], f_c, xf_l[i][:, lo:hi],
                        start=True, stop=True,
                    )
                    sc_t = sc_pool.tile([128, 512], f32, tag="sc")
                    col = c * 3 + j
                    nc.vector.scalar_tensor_tensor(
                        out=sc_t[:, 0:w],
                        in0=e_t[:, lo:hi],
                        scalar=1.0,
                        in1=z_t[:, 0:w],
                        op0=MULT,
                        op1=MULT,
                        accum_out=num3_l[i][:, col : col + 1],
                    )

            # image-major: image i's chunks run back-to-back, so only xe0
            # gates the stream start and epilogues/output DMAs for images
            # 0-2 overlap the stream instead of piling up at the end.
            # Each epilogue is DELAYED a few units past its image boundary:
            # emitted at the boundary, its PE-transpose gets scheduled ahead
            # of Z(39,i) on the in-order PE while depending on STT(39,i)
            # through the Pool chain — the near-circular wait stretches the
            # boundary ~2-3us. A few units later everything it needs is done
            # and its pieces slot into per-unit engine slack.
            EPI_DELAY_A = 3
            EPI_DELAY_B = 6
            epi_a, epi_b = [], []  # (due_unit_idx, image)
            unit = 0
            for i in range(IPC):
                for c in range(NCH):
                    pi, po = piece_of[c]
                    nq_c = nq_tiles[pi][:, po : po + 128]
                    f_c = f_tiles[pi][:, po : po + 128]
                    s_t = psS.tile([128, S], f32, tag="s")
                    for lo, hi in S_SLICES:
                        nc.tensor.matmul(
                            s_t[:, lo:hi], nq_c, xe_l[i][:, lo:hi],
                            start=True, stop=True,
                        )
                    e_t = e_pool.tile([128, S], bf16, tag="e")
                    nc.scalar.activation(
                        out=e_t, in_=s_t, func=EXP,
                        accum_out=den_l[i][:, c : c + 1],
                    )
                    if pending is not None:
                        emit_z(*pending)
                        if pending[2] == NCH - 1:
                            epi_a.append((unit + EPI_DELAY_A, pending[1]))
                            epi_b.append((unit + EPI_DELAY_B, pending[1]))
                    if epi_a and unit >= epi_a[0][0]:
                        emit_epilogue_a(epi_a.pop(0)[1])
                    if epi_b and unit >= epi_b[0][0]:
                        emit_epilogue_b(epi_b.pop(0)[1])
                    pending = (e_t, i, c, f_c)
                    unit += 1
            emit_z(*pending)
            for _, img in epi_a:
                emit_epilogue_a(img, on_dve=True)
            emit_epilogue_a(pending[1], on_dve=True)
            for _, img in epi_b:
                emit_epilogue_b(img)
            emit_epilogue_b(pending[1])

    nc.compile()
    return nc


def _get_nc():
    if "nc" not in _CACHE:
        _CACHE["nc"] = _build_nc()
    return _CACHE["nc"]


def _prep_in_maps(inputs):
    xf32 = np.asarray(inputs["x"], dtype=np.float32).reshape(N, C, S)
    pos32 = np.asarray(inputs["pos_emb"], dtype=np.float32).reshape(C, S)
    pos = pos32.astype(BF16)
    # On-device xf = xe - pos (bf16 subtract on GPSIMD) recovers bf16(x)
    # up to one extra bf16 rounding — well inside the error budget.
    xe = np.ascontiguousarray(
        (xf32.astype(BF16).astype(np.float32) + pos.astype(np.float32)).astype(
            BF16
        )
    )
    nqp = np.zeros((C, OUTP), BF16)
    nqp[:, :OUT] = (
        np.asarray(inputs["neuron_query"], dtype=np.float32)
        .reshape(C, OUT)
        .astype(BF16)
    )
    ftp = np.zeros((C, OUTP), BF16)
    ftp[:, :OUT] = (
        np.asarray(inputs["features"], dtype=np.float32)
        .reshape(C, OUT)
        .astype(BF16)
    )
    bias_pad = np.zeros(OUTP, np.float32)
    bias_pad[:OUT] = np.asarray(inputs["bias"], dtype=np.float32)
    # biasp[p, c] = bias_pad[c*128 + p]
    biasp = np.ascontiguousarray(bias_pad.reshape(NCH, 128).T)
    return [
        {
            "xe": np.ascontiguousarray(xe[i * IPC : (i + 1) * IPC]),
            "pos": pos,
            "neuron_query": nqp,
            "features": ftp,
            "bias": biasp,
        }
        for i in range(CORES)
    ]


def run_kernel(inputs, trace=False):
    """Returns (out [N, OUT] float32, BassKernelResults)."""
    from concourse.bass_utils import run_bass_kernel_spmd

    nc = _get_nc()
    in_maps = _prep_in_maps(inputs)
    res = run_bass_kernel_spmd(nc, in_maps, list(range(CORES)), trace=trace)
    out = np.concatenate([r["out"][:, :OUT] for r in res.results], axis=0)
    return np.asarray(out, dtype=np.float32), res


def kernel(**inputs):
    out, _ = run_kernel(inputs, trace=False)
    return out
